# revision 12
# baseline (speedup 1.0000x reference)
"""Trainium2 Bass kernel: causal multi-head attention block with LoRA (loralib-style).

Computes, for x:[4,2048,1024] (B,T,C), H=16 heads, D=64:
    qkv  = x @ Wqkv.T + bqkv + (x @ Aqkv.T) @ Bqkv.T * 2.0
    att  = causal_softmax(q k^T / sqrt(D))
    out  = att @ v   (per head), merged heads
    y    = out @ Wproj.T + bproj + (out @ Aproj.T) @ Bproj.T * 2.0

Sharding: 8 cores = (batch b in 0..3) x (head-group hg in 0..1, 8 heads each).
QKV is column-parallel (each core computes q,k,v only for its heads),
proj is row-parallel (each core computes a partial y over its heads'
features; host sums the two partials per batch). LoRA/bias are folded into
the matmuls as an extra K=9 contraction tile; the proj bias is split 50/50
between the two cores of a pair.

On-device dataflow is fully "transposed": host feeds x^T and pre-transposed
bf16 weights; S^T = K Q^T blocks (two heads packed in the PE array via row
groups), P^T = exp(S^T/8) (no max subtraction: logits are O(10)), causal via
diagonal-block trimming (columns below the diagonal are never computed) plus
one triangular mask multiply on the diagonal 128-block, O^T = V_aug P^T with
a ones-column in V_aug producing the softmax denominators for free.

Schedule: qb-major rounds (all 4 head pairs per q-block round) so that the
projection + output DMA for round r run during round r+1 instead of bunching
at the kernel tail. Startup: token-major x DMA slices + HAM-warmup matmuls.
"""

import os
import sys

import numpy as np

for _p in ("/opt/trn_rl_repo",):
    if _p not in sys.path and os.path.isdir(_p):
        sys.path.insert(0, _p)

import ml_dtypes
from contextlib import ExitStack

import concourse.bass as bass
import concourse.tile as tile
from concourse import bacc, mybir
from concourse.bass_utils import run_bass_kernel_spmd

BF16 = ml_dtypes.bfloat16
F32 = np.float32

B, T, C = 4, 2048, 1024
H, D = 16, 64
HL = 8            # heads per core
FQK = 2 * HL * D  # 1024 q+k features per core
FV = HL * D       # 512 v features per core
R = 8             # lora rank
SCALE = 2.0       # lora_alpha / lora_r
NCT = C // 128    # 8 contraction tiles over C
NTB = T // 512    # 4 token blocks of 512
NTC = T // 128    # 16 token chunks of 128
INV_SQRT_D = 1.0 / 8.0
NPAIR = HL // 2   # 4 packed head pairs per core

dt_bf16 = mybir.dt.bfloat16
dt_f32 = mybir.dt.float32

# module-level cache of the last run's results (exec_time_ns etc.)
LAST_RESULTS = None


def _build_program(nc, lora=True):
    """Emit the single-core SPMD program under a TileContext.

    lora=False omits the LoRA/bias contraction tiles entirely (used when the
    adapters and biases are all-zero, as with loralib's B=0 init).
    """
    # ---- DRAM I/O ----
    xT = nc.dram_tensor("xT", [C, T], dt_bf16, kind="ExternalInput").ap()
    wqkT = nc.dram_tensor("wqkT", [C, FQK], dt_bf16, kind="ExternalInput").ap()
    auga_qk = nc.dram_tensor("auga_qk", [R + 1, FQK], dt_bf16, kind="ExternalInput").ap()
    wvT = nc.dram_tensor("wvT", [C, FV], dt_bf16, kind="ExternalInput").ap()
    augb_v = nc.dram_tensor("augb_v", [R + 1, FV], dt_bf16, kind="ExternalInput").ap()
    aqkvT = nc.dram_tensor("aqkvT", [C, R], dt_bf16, kind="ExternalInput").ap()
    wpT = nc.dram_tensor("wpT", [FV, C], dt_bf16, kind="ExternalInput").ap()
    apT = nc.dram_tensor("apT", [FV, R], dt_bf16, kind="ExternalInput").ap()
    augb_p = nc.dram_tensor("augb_p", [R + 1, C], dt_bf16, kind="ExternalInput").ap()
    # one triangular diagonal-block mask, duplicated for the two packed heads
    masks = nc.dram_tensor("masks", [128, 256], dt_bf16, kind="ExternalInput").ap()
    out = nc.dram_tensor("out", [T, C], dt_bf16, kind="ExternalOutput").ap()

    with tile.TileContext(nc) as tc, ExitStack() as ctx:
        persist = ctx.enter_context(tc.tile_pool(name="persist", bufs=1))

        # ---- persistent SBUF tensors ----
        def chunk_views(dram_ap, n, m, dt, tag):
            big = persist.tile([128, n * m], dt, tag=tag, name=tag)
            src = dram_ap.rearrange("(a p) t -> p a t", p=128)    # [128, n, m]
            dst = big[:].rearrange("p (a t) -> p a t", a=n)
            return [big[:, i * m:(i + 1) * m] for i in range(n)], src, dst

        # x^T loaded in token-major slices so round-0 work starts before the
        # whole 4MB has landed
        xt_sb, xt_src, xt_dst = chunk_views(xT, NCT, T, dt_bf16, "xt")

        def xt_dma(tb):
            nc.sync.dma_start(out=xt_dst[:, :, tb * 512:(tb + 1) * 512],
                              in_=xt_src[:, :, tb * 512:(tb + 1) * 512])

        wqk_sb, wqk_src, wqk_dst = chunk_views(wqkT, NCT, FQK, dt_bf16, "wqk")
        wv_sb, wv_src, wv_dst = chunk_views(wvT, NCT, FV, dt_bf16, "wv")
        wp_sb, wp_src, wp_dst = chunk_views(wpT, FV // 128, C, dt_bf16, "wp")
        mask_sb = persist.tile([128, 256], dt_bf16, tag="mask", name="mask")
        tri3 = mask_sb[:].rearrange("p (h q) -> p h q", h=2)

        # DMA issue order = priority: round 0 needs xt slice 0 + wqk + wv
        xt_dma(0)
        for h in range(2):
            nc.sync.dma_start(out=wqk_dst[:, h * 4:(h + 1) * 4, :],
                              in_=wqk_src[:, h * 4:(h + 1) * 4, :])
        nc.sync.dma_start(out=wv_dst[:], in_=wv_src[:])
        xt_dma(1)
        xt_dma(2)
        xt_dma(3)
        nc.sync.dma_start(out=mask_sb[:], in_=masks[:, :])
        nc.sync.dma_start(out=wp_dst[:], in_=wp_src[:])

        aqkv_sb = None
        augaqk_sb = persist.tile([R + 1, FQK], dt_bf16, tag="augaqk")
        augbv_sb = persist.tile([R + 1, FV], dt_bf16, tag="augbv")
        augbp_sb = persist.tile([R + 1, C], dt_bf16, tag="augbp")
        ap_sb = None
        if lora:
            aqkv_sb, aq_src, aq_dst = chunk_views(aqkvT, NCT, R, dt_bf16, "aqkv")
            nc.sync.dma_start(out=aq_dst[:], in_=aq_src[:])
            nc.sync.dma_start(out=augaqk_sb[:], in_=auga_qk[:, :])
            nc.sync.dma_start(out=augbv_sb[:], in_=augb_v[:, :])
            ap_sb, ap_src, ap_dst = chunk_views(apT, FV // 128, R, dt_bf16, "ap")
            nc.sync.dma_start(out=ap_dst[:], in_=ap_src[:])
            nc.sync.dma_start(out=augbp_sb[:], in_=augb_p[:, :])

        # outputs of the QKV stage, all persistent in SBUF
        qk_sb = [persist.tile([128, T], dt_bf16, tag=f"qk{i}", name=f"qk{i}")
                 for i in range(FQK // 128)]
        # v in natural orientation, with a ones column per head: [t,(h,65)]
        vaug_sb = [persist.tile([128, HL * (D + 1)], dt_bf16, tag=f"vaug{i}", name=f"vaug{i}")
                   for i in range(NTC)]
        # normalized attention outputs, transposed: [f_local, t]
        ot_sb = [persist.tile([128, T], dt_bf16, tag=f"ot{i}", name=f"ot{i}")
                 for i in range(FV // 128)]
        # lora intermediates as matmul k-tiles: rows 0..7 = v^T/u^T, row 8 = ones
        rhs_aug = persist.tile([R + 1, T], dt_bf16, tag="rhs_aug")
        u_aug = persist.tile([R + 1, T], dt_bf16, tag="u_aug")
        if lora:
            nc.vector.memset(rhs_aug[:], 1.0)
            nc.vector.memset(u_aug[:], 1.0)

        # HAM warm-up scratch: keep the PE busy during the input-DMA wait so
        # the clock gate is at 8/8 when real matmuls start
        warm_sb = persist.tile([128, 512], dt_bf16, tag="warm", name="warm")
        nc.vector.memset(warm_sb[:], 0.25)

        sb_pt = ctx.enter_context(tc.tile_pool(name="pt", bufs=6))
        sb_nrm = ctx.enter_context(tc.tile_pool(name="nrm", bufs=4))
        sb_stg = ctx.enter_context(tc.tile_pool(name="stg", bufs=4))
        sb_y = ctx.enter_context(tc.tile_pool(name="ysb", bufs=3))
        with tc.tile_pool(name="psAll", bufs=2, space="PSUM") as ps:

            # ---- HAM warm-up matmuls (no data deps; run during DMA wait) ----
            for i in range(40):
                if i % 8 == 0:
                    warm_ps = ps.tile([128, 512], dt_f32, tag="pm", name="warm")
                nc.tensor.matmul(warm_ps[:], warm_sb[:, 0:128], warm_sb[:],
                                 start=(i % 8 == 0), stop=(i % 8 == 7))

            # ---- filler blocks (qkv / v / proj), as fine-grained thunks ----
            emitted = set()

            def lora_v_block(tb):
                """v^T = (x @ Aqkv.T)^T for one token block (lora only)."""
                pv = ps.tile([R, 512], dt_f32, tag="pm", name="pv")
                for ct in range(NCT):
                    nc.tensor.matmul(
                        pv[:], aqkv_sb[ct][:], xt_sb[ct][:, tb * 512:(tb + 1) * 512],
                        start=(ct == 0), stop=(ct == NCT - 1))
                nc.vector.tensor_copy(rhs_aug[0:R, tb * 512:(tb + 1) * 512], pv[:])

            def qk_thunks(fc, tb):
                """one [128, 512] block of qk^T[f, t], split into 4 thunks."""
                ref = {}

                def part(ph, fc=fc, tb=tb):
                    def f():
                        if ph == 0:
                            ref['pm'] = ps.tile([128, 512], dt_f32, tag="pm",
                                                name="pm")
                        pm = ref['pm']
                        for ct in (2 * ph, 2 * ph + 1):
                            nc.tensor.matmul(
                                pm[:],
                                wqk_sb[ct][:, fc * 128:(fc + 1) * 128],
                                xt_sb[ct][:, tb * 512:(tb + 1) * 512],
                                start=(ct == 0),
                                stop=(not lora and ct == NCT - 1))
                        if ph == 3:
                            if lora:
                                nc.tensor.matmul(
                                    pm[:],
                                    augaqk_sb[:, fc * 128:(fc + 1) * 128],
                                    rhs_aug[:, tb * 512:(tb + 1) * 512],
                                    start=False, stop=True)
                            nc.vector.tensor_copy(
                                qk_sb[fc][:, tb * 512:(tb + 1) * 512], pm[:])
                            emitted.add(("qk", fc, tb))
                    return f
                return [part(i) for i in range(4)]

            def v_thunks(ti):
                """v (natural orientation + ones cols) for one 128-chunk."""
                ref = {}

                def part(ph, ti=ti):
                    def f():
                        if ph == 0:
                            ref['pm'] = ps.tile([128, 512], dt_f32, tag="pm",
                                                name="pm")
                        pm = ref['pm']
                        for ct in (4 * ph, 4 * ph + 1, 4 * ph + 2, 4 * ph + 3):
                            nc.tensor.matmul(
                                pm[:],
                                xt_sb[ct][:, ti * 128:(ti + 1) * 128],
                                wv_sb[ct][:],
                                start=(ct == 0),
                                stop=(not lora and ct == NCT - 1))
                        if ph == 1:
                            if lora:
                                nc.tensor.matmul(
                                    pm[:],
                                    rhs_aug[:, ti * 128:(ti + 1) * 128],
                                    augbv_sb[:],
                                    start=False, stop=True)
                            v3 = vaug_sb[ti].rearrange("p (h e) -> p h e", h=HL)
                            nc.vector.tensor_copy(
                                v3[:, :, 0:D],
                                pm[:].rearrange("p (h e) -> p h e", h=HL))
                            nc.vector.memset(v3[:, :, D:D + 1], 1.0)
                            emitted.add(("v", ti))
                    return f
                return [part(0), part(1)]

            def u_block(tb):
                """u^T = (o_norm @ Aproj_local.T)^T (lora only)."""
                pu = ps.tile([R, 512], dt_f32, tag="pm", name="pu")
                for fc in range(FV // 128):
                    nc.tensor.matmul(
                        pu[:], ap_sb[fc][:], ot_sb[fc][:, tb * 512:(tb + 1) * 512],
                        start=(fc == 0), stop=(fc == FV // 128 - 1))
                nc.vector.tensor_copy(u_aug[0:R, tb * 512:(tb + 1) * 512], pu[:])

            def y_thunks(ti, tags=("pm", "pm")):
                """partial projection output for one token chunk: 2 eb thunks
                + final DMA. Output in bf16 (host accumulates in f32)."""
                ref = {}

                def part(eb, ti=ti, tags=tags):
                    def f():
                        if eb == 0:
                            ref['ys'] = sb_y.tile([128, C], dt_bf16, tag="ys",
                                                  name="ys")
                        ys = ref['ys']
                        py = ps.tile([128, 512], dt_f32, tag=tags[eb], name="py",
                                     bufs=1 if tags[eb] != "pm" else None)
                        for fc in range(FV // 128):
                            nc.tensor.matmul(
                                py[:],
                                ot_sb[fc][:, ti * 128:(ti + 1) * 128],
                                wp_sb[fc][:, eb * 512:(eb + 1) * 512],
                                start=(fc == 0),
                                stop=(not lora and fc == FV // 128 - 1))
                        if lora:
                            nc.tensor.matmul(
                                py[:],
                                u_aug[:, ti * 128:(ti + 1) * 128],
                                augbp_sb[:, eb * 512:(eb + 1) * 512],
                                start=False, stop=True)
                        nc.vector.tensor_copy(ys[:, eb * 512:(eb + 1) * 512],
                                              py[:])
                        if eb == 1:
                            nc.sync.dma_start(
                                out=out[ti * 128:(ti + 1) * 128, :], in_=ys[:])
                    return f
                return [part(0), part(1)]

            # ---- attention emission: flat software pipeline over all
            # (head pair, q-block, k-chunk) steps, qb-major. S/exp for step
            # i+1 is emitted before O for step i, including across segment
            # boundaries, so the scalar engine's exp stream never drains.
            qmust = []   # qk/v blocks, in need-order (ensure() pops these)
            qproj = []   # projection/u blocks, interleaved by pump()
            nrm_q = []
            ptog = [0]

            def pump(n=1):
                for _ in range(n):
                    ptog[0] ^= 1
                    src = (qproj if (qproj and (ptog[0] or not qmust))
                           else qmust)
                    if src:
                        src.pop(0)()

            def ensure(*keys):
                for k in keys:
                    while k not in emitted:
                        assert qmust, f"dependency {k} not in queue"
                        qmust.pop(0)()

            def emit_S(hp, qb, kb):
                q_ch = qk_sb[hp]        # rows 0-63 head 2hp, 64-127 head 2hp+1
                k_ch = qk_sb[NPAIR + hp]
                j = kb - 4 * qb
                lo = 128 * j if j > 0 else 0
                s = ps.tile([128, 1024], dt_f32, tag="S", name="S")
                nc.tensor.matmul(
                    s[:, lo:512],
                    k_ch[0:64, kb * 128:(kb + 1) * 128],
                    q_ch[0:64, qb * 512 + lo:(qb + 1) * 512],
                    start=True, stop=True)
                nc.tensor.matmul(
                    s[:, 512 + lo:1024],
                    k_ch[64:128, kb * 128:(kb + 1) * 128],
                    q_ch[64:128, qb * 512 + lo:(qb + 1) * 512],
                    start=True, stop=True)
                pt = sb_pt.tile([128, 1024], dt_bf16, tag="PT")
                pt3 = pt[:].rearrange("p (h q) -> p h q", h=2)
                s3 = s[:].rearrange("p (h q) -> p h q", h=2)
                if lo:
                    nc.scalar.activation(
                        pt3[:, :, lo:512], s3[:, :, lo:512],
                        mybir.ActivationFunctionType.Exp, scale=INV_SQRT_D)
                else:
                    nc.scalar.activation(
                        pt[:], s[:], mybir.ActivationFunctionType.Exp,
                        scale=INV_SQRT_D)
                if j >= 0:  # triangular mask on the diagonal 128-block
                    nc.vector.tensor_mul(
                        pt3[:, :, lo:lo + 128], pt3[:, :, lo:lo + 128],
                        tri3[:])
                return pt, lo

            seg_o = {}

            def emit_O(hp, qb, kb, pt, lo):
                nkb = 4 * qb + 4
                if kb == 0:
                    seg_o[(hp, qb)] = (
                        ps.tile([D + 1, 512], dt_f32, tag="o0", name="o0",
                                bufs=1),
                        ps.tile([D + 1, 512], dt_f32, tag="o1", name="o1",
                                bufs=1))
                o0, o1 = seg_o[(hp, qb)]
                v3 = vaug_sb[kb]
                nc.tensor.matmul(
                    o0[:, lo:512],
                    v3[:, (2 * hp) * (D + 1):(2 * hp + 1) * (D + 1)],
                    pt[:, lo:512],
                    start=(kb == 0), stop=(kb == nkb - 1))
                nc.tensor.matmul(
                    o1[:, lo:512],
                    v3[:, (2 * hp + 1) * (D + 1):(2 * hp + 2) * (D + 1)],
                    pt[:, 512 + lo:1024],
                    start=(kb == 0), stop=(kb == nkb - 1))

            def emit_segend(hp, qb):
                # evict unnormalized O (one head on ScalarE, one on VectorE),
                # gather the denominator psum rows, one batched fast
                # reciprocal; broadcast+multiply deferred via nrm_q
                o0, o1 = seg_o.pop((hp, qb))
                coll = sb_nrm.tile([33, 512], dt_f32, tag="coll", name="coll")
                nc.vector.memset(coll[:], 1.0)
                stgs = []
                for h01, o in ((0, o0), (1, o1)):
                    stg = sb_stg.tile([D, 512], dt_bf16, tag="stg", name="stg")
                    if h01 == 0:
                        nc.scalar.copy(stg[:], o[0:D, :])
                    else:
                        nc.vector.tensor_copy(stg[:], o[0:D, :])
                    nc.vector.tensor_copy(coll[32 * h01:32 * h01 + 1, :],
                                          o[D:D + 1, :])
                    stgs.append(stg)
                rcp = sb_nrm.tile([33, 512], dt_f32, tag="rcp", name="rcp")
                nc.vector.reciprocal_approx_fast(rcp[:], coll[:])

                def nrm_thunk(hp=hp, qb=qb, stgs=stgs, rcp=rcp):
                    for h01 in range(2):
                        rrow = sb_nrm.tile([1, 512], dt_bf16, tag="rrow",
                                           name="rrow")
                        nc.vector.tensor_copy(rrow[:],
                                              rcp[32 * h01:32 * h01 + 1, :])
                        rbc = sb_nrm.tile([64, 512], dt_bf16, tag="rbc",
                                          name="rbc")
                        nc.gpsimd.partition_broadcast(rbc[:], rrow[:])
                        nc.vector.tensor_mul(
                            ot_sb[hp][h01 * 64:(h01 + 1) * 64,
                                      qb * 512:(qb + 1) * 512],
                            stgs[h01][:], rbc[:])
                nrm_q.append(nrm_thunk)

            # ---- fill the filler queue in need-order ----
            qmust += [th for ti in range(4) for th in v_thunks(ti)]
            qmust += qk_thunks(1, 0) + qk_thunks(NPAIR + 1, 0)
            qmust += qk_thunks(2, 0) + qk_thunks(NPAIR + 2, 0)
            qmust += qk_thunks(3, 0) + qk_thunks(NPAIR + 3, 0)
            for tb in range(1, NTB):
                qmust += qk_thunks(0, tb) + qk_thunks(NPAIR, tb)
                qmust += v_thunks(4 * tb) + v_thunks(4 * tb + 1)
                qmust += v_thunks(4 * tb + 2) + v_thunks(4 * tb + 3)
                qmust += qk_thunks(1, tb) + qk_thunks(NPAIR + 1, tb)
                qmust += qk_thunks(2, tb) + qk_thunks(NPAIR + 2, tb)
                qmust += qk_thunks(3, tb) + qk_thunks(NPAIR + 3, tb)

            if lora:
                for tb in range(NTB):
                    lora_v_block(tb)
            for th in qk_thunks(0, 0) + qk_thunks(NPAIR, 0):
                th()

            steps = [(hp, qb, kb)
                     for qb in range(NTB)
                     for hp in range(NPAIR)
                     for kb in range(4 * qb + 4)]

            pend = None          # (hp, qb, kb, pt, lo) awaiting its O
            for idx, (hp, qb, kb) in enumerate(steps):
                if kb == 0:
                    ensure(("qk", hp, qb), ("qk", NPAIR + hp, qb),
                           ("v", 4 * qb + 3))
                pt, lo = emit_S(hp, qb, kb)
                if pend is not None:
                    phk, pqb, pkb, ppt, plo = pend
                    emit_O(phk, pqb, pkb, ppt, plo)
                    if pkb == 4 * pqb + 3:
                        emit_segend(phk, pqb)
                if hp == 0 and kb == 0 and qb > 0:
                    # round qb starts (previous round's last segend just
                    # emitted above): flush round qb-1's normalize, then
                    # queue its projection (reads normalized ot columns)
                    while nrm_q:
                        nrm_q.pop(0)()
                    if lora:
                        qproj.append(lambda tb=qb - 1: u_block(tb))
                    for ti in range(4 * (qb - 1), 4 * qb):
                        qproj += y_thunks(ti)
                if nrm_q and kb == 1:
                    nrm_q.pop(0)()
                pump(2 if len(qmust) + len(qproj) >
                     (len(steps) - idx) else 1)
                pend = (hp, qb, kb, pt, lo)
            phk, pqb, pkb, ppt, plo = pend
            emit_O(phk, pqb, pkb, ppt, plo)
            emit_segend(phk, pqb)

            # tail: last round's normalize + projection
            while nrm_q:
                nrm_q.pop(0)()
            if lora:
                u_block(NTB - 1)
            for i, ti in enumerate(range(4 * (NTB - 1), 4 * NTB)):
                tags = ("o0", "o1") if i >= 2 else ("pm", "pm")
                for th in y_thunks(ti, tags=tags):
                    th()
            while qmust or qproj:
                pump(1)


def build_bass(lora=True):
    nc = bacc.Bacc("TRN2", target_bir_lowering=False, debug=False, num_devices=8)
    _build_program(nc, lora=lora)
    nc.compile()
    return nc


def prepare_core_inputs(x, Wqkv, bqkv, Aqkv, Bqkv, Wproj, bproj, Aproj, Bproj):
    """Shard + lay out inputs for the 8 cores. Core c = (b = c//2, hg = c%2)."""
    def b16(a):
        return np.ascontiguousarray(a, dtype=np.float32).astype(BF16)

    # one triangular diagonal-block mask (same for every crossing offset),
    # duplicated for the two packed heads: mask[k', u] = (u >= k')
    kk = np.arange(128)[:, None]
    uu = np.arange(128)[None, :]
    m = (uu >= kk).astype(np.float32)
    masks = b16(np.concatenate([m, m], axis=1))  # [128, 256]

    aqkvT = b16(Aqkv.T)                      # [C, R]

    in_maps = []
    for c in range(8):
        bb, hg = c // 2, c % 2
        fsl = slice(hg * FV, (hg + 1) * FV)  # local head feature slice
        q_rows = slice(hg * FV, (hg + 1) * FV)
        k_rows = slice(C + hg * FV, C + (hg + 1) * FV)
        v_rows = slice(2 * C + hg * FV, 2 * C + (hg + 1) * FV)

        Wqk = np.concatenate([Wqkv[q_rows], Wqkv[k_rows]], axis=0)   # [FQK, C]
        Bqk = np.concatenate([Bqkv[q_rows], Bqkv[k_rows]], axis=0)   # [FQK, R]
        bqk = np.concatenate([bqkv[q_rows], bqkv[k_rows]], axis=0)   # [FQK]

        in_maps.append({
            "xT": b16(np.asarray(x)[bb].T),
            "wqkT": b16(Wqk.T),
            "auga_qk": b16(np.concatenate(
                [SCALE * Bqk.T, bqk[None, :]], axis=0)),
            "wvT": b16(Wqkv[v_rows].T),
            "augb_v": b16(np.concatenate(
                [SCALE * Bqkv[v_rows].T, bqkv[v_rows][None, :]], axis=0)),
            "aqkvT": aqkvT,
            "wpT": b16(Wproj[:, fsl].T),
            "apT": b16(Aproj[:, fsl].T),
            "augb_p": b16(np.concatenate(
                [SCALE * Bproj.T, 0.5 * bproj[None, :]], axis=0)),
            "masks": masks,
        })
    return in_maps


_CACHED_NC = None
TRACE = False  # set True (e.g. from test.py) to request an NTFF-profiled run


def _install_axon_ntff_hook():
    """Provide antenv.axon_hooks (NTFF profiling hook) if the image lacks it.

    Mirrors trn_agent_boot.trn_boot._ntff_profile_via_ctypes: drives NRT
    profiling on the axon terminal via the libaxon_pjrt.so C ABI.
    """
    try:
        from antenv.axon_hooks import get_axon_ntff_profile_hook  # noqa: F401
        return
    except ImportError:
        pass
    import contextlib
    import ctypes
    import types

    import antenv

    so_path = "/opt/axon/libaxon_pjrt.so"
    hook = None
    if os.path.exists(so_path):
        lib = ctypes.CDLL(so_path)
        if hasattr(lib, "axon_start_nrt_profile"):
            lib.axon_start_nrt_profile.argtypes = [
                ctypes.POINTER(ctypes.c_int64), ctypes.c_size_t]
            lib.axon_start_nrt_profile.restype = ctypes.c_int64
            lib.axon_stop_nrt_profile.argtypes = [ctypes.c_char_p]
            lib.axon_stop_nrt_profile.restype = ctypes.c_int64

            @contextlib.contextmanager
            def _hook(output_dir, device_ids):
                import jax
                jax.devices()
                if device_ids:
                    ids = (ctypes.c_int64 * len(device_ids))(*device_ids)
                    rc = lib.axon_start_nrt_profile(ids, len(device_ids))
                else:
                    rc = lib.axon_start_nrt_profile(None, 0)
                if rc != 0:
                    raise RuntimeError(f"axon_start_nrt_profile rc={rc}")
                try:
                    yield
                finally:
                    n = lib.axon_stop_nrt_profile(str(output_dir).encode())
                    print(f"ntff profile: {n} file(s) -> {output_dir}",
                          file=sys.stderr)

            hook = _hook

    mod = types.ModuleType("antenv.axon_hooks")
    state = {"h": hook}
    mod.get_axon_ntff_profile_hook = lambda: state["h"]
    mod.set_axon_ntff_profile_hook = lambda h: state.update(h=h)
    sys.modules["antenv.axon_hooks"] = mod
    antenv.axon_hooks = mod


def kernel(**inputs):
    global _CACHED_NC, LAST_RESULTS
    in_maps = prepare_core_inputs(**inputs)
    # loralib initializes B to zero, and the biases here are zero: when every
    # adapter/bias contribution is exactly zero, the extra contraction tiles
    # are mathematically a no-op — use the leaner program variant.
    lora = any(
        np.any(np.asarray(inputs[k]) != 0)
        for k in ("Bqkv", "Bproj", "bqkv", "bproj"))
    if _CACHED_NC is None:
        _CACHED_NC = build_bass(lora=lora)
    if TRACE:
        _install_axon_ntff_hook()
    res = run_bass_kernel_spmd(
        _CACHED_NC, in_maps, core_ids=list(range(8)), trace=TRACE,
    )
    LAST_RESULTS = res
    y = np.zeros((B, T, C), dtype=np.float32)
    for c in range(8):
        y[c // 2] += np.asarray(res.results[c]["out"]).astype(np.float32)
    return y


# revision 14
# speedup vs baseline: 1.2470x; 1.2470x over previous
"""Trainium2 Bass kernel: causal multi-head attention block with LoRA (loralib-style).

Computes, for x:[4,2048,1024] (B,T,C), H=16 heads, D=64:
    qkv  = x @ Wqkv.T + bqkv + (x @ Aqkv.T) @ Bqkv.T * 2.0
    att  = causal_softmax(q k^T / sqrt(D))
    out  = att @ v   (per head), merged heads
    y    = out @ Wproj.T + bproj + (out @ Aproj.T) @ Bproj.T * 2.0

Sharding: 8 cores = (batch b in 0..3) x (head-group hg in 0..1, 8 heads each).
QKV is column-parallel (each core computes q,k,v only for its heads),
proj is row-parallel (each core computes a partial y over its heads'
features; host sums the two partials per batch). LoRA/bias are folded into
the matmuls as an extra K=9 contraction tile; the proj bias is split 50/50
between the two cores of a pair.

On-device dataflow is fully "transposed": host feeds x^T and pre-transposed
bf16 weights; S^T = K Q^T blocks (two heads packed in the PE array via row
groups), P^T = exp(S^T/8) (no max subtraction: logits are O(10)), causal via
diagonal-block trimming (columns below the diagonal are never computed) plus
one triangular mask multiply on the diagonal 128-block, O^T = V_aug P^T with
a ones-column in V_aug producing the softmax denominators for free.

Schedule: qb-major rounds (all 4 head pairs per q-block round) so that the
projection + output DMA for round r run during round r+1 instead of bunching
at the kernel tail. Startup: token-major x DMA slices + HAM-warmup matmuls.
"""

import os
import sys

import numpy as np

for _p in ("/opt/trn_rl_repo",):
    if _p not in sys.path and os.path.isdir(_p):
        sys.path.insert(0, _p)

import ml_dtypes
from contextlib import ExitStack

import concourse.bass as bass
import concourse.tile as tile
from concourse import bacc, mybir
from concourse.bass_utils import run_bass_kernel_spmd

BF16 = ml_dtypes.bfloat16
F32 = np.float32

B, T, C = 4, 2048, 1024
H, D = 16, 64
HL = 8            # heads per core
FQK = 2 * HL * D  # 1024 q+k features per core
FV = HL * D       # 512 v features per core
R = 8             # lora rank
SCALE = 2.0       # lora_alpha / lora_r
NCT = C // 128    # 8 contraction tiles over C
NTB = T // 512    # 4 token blocks of 512
NTC = T // 128    # 16 token chunks of 128
INV_SQRT_D = 1.0 / 8.0
NPAIR = HL // 2   # 4 packed head pairs per core

dt_bf16 = mybir.dt.bfloat16
dt_f32 = mybir.dt.float32

# module-level cache of the last run's results (exec_time_ns etc.)
LAST_RESULTS = None


def _build_program(nc, lora=True):
    """Emit the single-core SPMD program under a TileContext.

    lora=False omits the LoRA/bias contraction tiles entirely (used when the
    adapters and biases are all-zero, as with loralib's B=0 init).
    """
    # ---- DRAM I/O ----
    xT = nc.dram_tensor("xT", [C, T], dt_bf16, kind="ExternalInput").ap()
    wqkT = nc.dram_tensor("wqkT", [C, FQK], dt_bf16, kind="ExternalInput").ap()
    auga_qk = nc.dram_tensor("auga_qk", [R + 1, FQK], dt_bf16, kind="ExternalInput").ap()
    wvT = nc.dram_tensor("wvT", [C, FV], dt_bf16, kind="ExternalInput").ap()
    augb_v = nc.dram_tensor("augb_v", [R + 1, FV], dt_bf16, kind="ExternalInput").ap()
    aqkvT = nc.dram_tensor("aqkvT", [C, R], dt_bf16, kind="ExternalInput").ap()
    wpT = nc.dram_tensor("wpT", [FV, C], dt_bf16, kind="ExternalInput").ap()
    apT = nc.dram_tensor("apT", [FV, R], dt_bf16, kind="ExternalInput").ap()
    augb_p = nc.dram_tensor("augb_p", [R + 1, C], dt_bf16, kind="ExternalInput").ap()
    # one triangular diagonal-block mask, duplicated for the two packed heads
    masks = nc.dram_tensor("masks", [128, 256], dt_bf16, kind="ExternalInput").ap()
    out = nc.dram_tensor("out", [T, C], dt_bf16, kind="ExternalOutput").ap()

    with tile.TileContext(nc) as tc, ExitStack() as ctx:
        persist = ctx.enter_context(tc.tile_pool(name="persist", bufs=1))

        # ---- persistent SBUF tensors ----
        def chunk_views(dram_ap, n, m, dt, tag):
            big = persist.tile([128, n * m], dt, tag=tag, name=tag)
            src = dram_ap.rearrange("(a p) t -> p a t", p=128)    # [128, n, m]
            dst = big[:].rearrange("p (a t) -> p a t", a=n)
            return [big[:, i * m:(i + 1) * m] for i in range(n)], src, dst

        # x^T loaded in token-major slices so round-0 work starts before the
        # whole 4MB has landed
        xt_sb, xt_src, xt_dst = chunk_views(xT, NCT, T, dt_bf16, "xt")

        def xt_dma(tb):
            nc.sync.dma_start(out=xt_dst[:, :, tb * 512:(tb + 1) * 512],
                              in_=xt_src[:, :, tb * 512:(tb + 1) * 512])

        wqk_sb, wqk_src, wqk_dst = chunk_views(wqkT, NCT, FQK, dt_bf16, "wqk")
        wv_sb, wv_src, wv_dst = chunk_views(wvT, NCT, FV, dt_bf16, "wv")
        wp_sb, wp_src, wp_dst = chunk_views(wpT, FV // 128, C, dt_bf16, "wp")
        mask_sb = persist.tile([128, 256], dt_bf16, tag="mask", name="mask")
        tri3 = mask_sb[:].rearrange("p (h q) -> p h q", h=2)

        # DMA issue order = priority: round 0 needs xt slice 0 + wqk + wv
        xt_dma(0)
        for h in range(2):
            nc.sync.dma_start(out=wqk_dst[:, h * 4:(h + 1) * 4, :],
                              in_=wqk_src[:, h * 4:(h + 1) * 4, :])
        nc.sync.dma_start(out=wv_dst[:], in_=wv_src[:])
        xt_dma(1)
        xt_dma(2)
        xt_dma(3)
        nc.sync.dma_start(out=mask_sb[:], in_=masks[:, :])
        nc.sync.dma_start(out=wp_dst[:], in_=wp_src[:])

        aqkv_sb = None
        augaqk_sb = persist.tile([R + 1, FQK], dt_bf16, tag="augaqk")
        augbv_sb = persist.tile([R + 1, FV], dt_bf16, tag="augbv")
        augbp_sb = persist.tile([R + 1, C], dt_bf16, tag="augbp")
        ap_sb = None
        if lora:
            aqkv_sb, aq_src, aq_dst = chunk_views(aqkvT, NCT, R, dt_bf16, "aqkv")
            nc.sync.dma_start(out=aq_dst[:], in_=aq_src[:])
            nc.sync.dma_start(out=augaqk_sb[:], in_=auga_qk[:, :])
            nc.sync.dma_start(out=augbv_sb[:], in_=augb_v[:, :])
            ap_sb, ap_src, ap_dst = chunk_views(apT, FV // 128, R, dt_bf16, "ap")
            nc.sync.dma_start(out=ap_dst[:], in_=ap_src[:])
            nc.sync.dma_start(out=augbp_sb[:], in_=augb_p[:, :])

        # outputs of the QKV stage, all persistent in SBUF
        qk_sb = [persist.tile([128, T], dt_bf16, tag=f"qk{i}", name=f"qk{i}")
                 for i in range(FQK // 128)]
        # v in natural orientation, with a ones column per head: [t,(h,65)]
        vaug_sb = [persist.tile([128, HL * (D + 1)], dt_bf16, tag=f"vaug{i}", name=f"vaug{i}")
                   for i in range(NTC)]
        # normalized attention outputs, transposed: [f_local, t]
        ot_sb = [persist.tile([128, T], dt_bf16, tag=f"ot{i}", name=f"ot{i}")
                 for i in range(FV // 128)]
        # lora intermediates as matmul k-tiles: rows 0..7 = v^T/u^T, row 8 = ones
        rhs_aug = persist.tile([R + 1, T], dt_bf16, tag="rhs_aug")
        u_aug = persist.tile([R + 1, T], dt_bf16, tag="u_aug")
        if lora:
            nc.vector.memset(rhs_aug[:], 1.0)
            nc.vector.memset(u_aug[:], 1.0)

        # HAM warm-up scratch: keep the PE busy during the input-DMA wait so
        # the clock gate is at 8/8 when real matmuls start
        warm_sb = persist.tile([128, 512], dt_bf16, tag="warm", name="warm")
        nc.vector.memset(warm_sb[:], 0.25)

        sb_pt = ctx.enter_context(tc.tile_pool(name="pt", bufs=6))
        sb_nrm = ctx.enter_context(tc.tile_pool(name="nrm", bufs=4))
        sb_stg = ctx.enter_context(tc.tile_pool(name="stg", bufs=4))
        sb_y = ctx.enter_context(tc.tile_pool(name="ysb", bufs=3))
        with tc.tile_pool(name="psAll", bufs=2, space="PSUM") as ps:

            # ---- HAM warm-up matmuls (no data deps; run during DMA wait) ----
            for i in range(40):
                if i % 8 == 0:
                    warm_ps = ps.tile([128, 512], dt_f32, tag="pm", name="warm")
                nc.tensor.matmul(warm_ps[:], warm_sb[:, 0:128], warm_sb[:],
                                 start=(i % 8 == 0), stop=(i % 8 == 7))

            # ---- filler blocks (qkv / v / proj), as fine-grained thunks ----
            emitted = set()

            def lora_v_block(tb):
                """v^T = (x @ Aqkv.T)^T for one token block (lora only)."""
                pv = ps.tile([R, 512], dt_f32, tag="pm", name="pv")
                for ct in range(NCT):
                    nc.tensor.matmul(
                        pv[:], aqkv_sb[ct][:], xt_sb[ct][:, tb * 512:(tb + 1) * 512],
                        start=(ct == 0), stop=(ct == NCT - 1))
                nc.vector.tensor_copy(rhs_aug[0:R, tb * 512:(tb + 1) * 512], pv[:])

            def qk_thunks(fc, tb):
                """one [128, 512] block of qk^T[f, t], split into 4 thunks."""
                ref = {}

                def part(ph, fc=fc, tb=tb):
                    def f():
                        if ph == 0:
                            ref['pm'] = ps.tile([128, 512], dt_f32, tag="pm",
                                                name="pm")
                        pm = ref['pm']
                        for ct in (2 * ph, 2 * ph + 1):
                            nc.tensor.matmul(
                                pm[:],
                                wqk_sb[ct][:, fc * 128:(fc + 1) * 128],
                                xt_sb[ct][:, tb * 512:(tb + 1) * 512],
                                start=(ct == 0),
                                stop=(not lora and ct == NCT - 1))
                        if ph == 3:
                            if lora:
                                nc.tensor.matmul(
                                    pm[:],
                                    augaqk_sb[:, fc * 128:(fc + 1) * 128],
                                    rhs_aug[:, tb * 512:(tb + 1) * 512],
                                    start=False, stop=True)
                            nc.vector.tensor_copy(
                                qk_sb[fc][:, tb * 512:(tb + 1) * 512], pm[:])
                            emitted.add(("qk", fc, tb))
                    return f
                return [part(i) for i in range(4)]

            def v_thunks(ti):
                """v (natural orientation + ones cols) for one 128-chunk."""
                ref = {}

                def part(ph, ti=ti):
                    def f():
                        if ph == 0:
                            ref['pm'] = ps.tile([128, 512], dt_f32, tag="pm",
                                                name="pm")
                        pm = ref['pm']
                        for ct in (4 * ph, 4 * ph + 1, 4 * ph + 2, 4 * ph + 3):
                            nc.tensor.matmul(
                                pm[:],
                                xt_sb[ct][:, ti * 128:(ti + 1) * 128],
                                wv_sb[ct][:],
                                start=(ct == 0),
                                stop=(not lora and ct == NCT - 1))
                        if ph == 1:
                            if lora:
                                nc.tensor.matmul(
                                    pm[:],
                                    rhs_aug[:, ti * 128:(ti + 1) * 128],
                                    augbv_sb[:],
                                    start=False, stop=True)
                            v3 = vaug_sb[ti].rearrange("p (h e) -> p h e", h=HL)
                            nc.vector.tensor_copy(
                                v3[:, :, 0:D],
                                pm[:].rearrange("p (h e) -> p h e", h=HL))
                            nc.vector.memset(v3[:, :, D:D + 1], 1.0)
                            emitted.add(("v", ti))
                    return f
                return [part(0), part(1)]

            def u_block(tb):
                """u^T = (o_norm @ Aproj_local.T)^T (lora only)."""
                pu = ps.tile([R, 512], dt_f32, tag="pm", name="pu")
                for fc in range(FV // 128):
                    nc.tensor.matmul(
                        pu[:], ap_sb[fc][:], ot_sb[fc][:, tb * 512:(tb + 1) * 512],
                        start=(fc == 0), stop=(fc == FV // 128 - 1))
                nc.vector.tensor_copy(u_aug[0:R, tb * 512:(tb + 1) * 512], pu[:])

            def y_thunks(ti, tags=("pm", "pm")):
                """partial projection output for one token chunk: 2 eb thunks
                + final DMA. Output in bf16 (host accumulates in f32)."""
                ref = {}

                def part(eb, ti=ti, tags=tags):
                    def f():
                        if eb == 0:
                            ref['ys'] = sb_y.tile([128, C], dt_bf16, tag="ys",
                                                  name="ys")
                        ys = ref['ys']
                        py = ps.tile([128, 512], dt_f32, tag=tags[eb], name="py",
                                     bufs=1 if tags[eb] != "pm" else None)
                        for fc in range(FV // 128):
                            nc.tensor.matmul(
                                py[:],
                                ot_sb[fc][:, ti * 128:(ti + 1) * 128],
                                wp_sb[fc][:, eb * 512:(eb + 1) * 512],
                                start=(fc == 0),
                                stop=(not lora and fc == FV // 128 - 1))
                        if lora:
                            nc.tensor.matmul(
                                py[:],
                                u_aug[:, ti * 128:(ti + 1) * 128],
                                augbp_sb[:, eb * 512:(eb + 1) * 512],
                                start=False, stop=True)
                        nc.vector.tensor_copy(ys[:, eb * 512:(eb + 1) * 512],
                                              py[:])
                        if eb == 1:
                            nc.sync.dma_start(
                                out=out[ti * 128:(ti + 1) * 128, :], in_=ys[:])
                    return f
                return [part(0), part(1)]

            # ---- attention emission: flat software pipeline over all
            # (head pair, q-block, k-chunk) steps, qb-major. S/exp for step
            # i+1 is emitted before O for step i, including across segment
            # boundaries, so the scalar engine's exp stream never drains.
            qmust = []   # qk/v blocks, in need-order (ensure() pops these)
            qproj = []   # projection/u blocks, interleaved by pump()
            nrm_q = []
            ptog = [0]

            def pump(n=1):
                for _ in range(n):
                    ptog[0] ^= 1
                    src = (qproj if (qproj and (ptog[0] or not qmust))
                           else qmust)
                    if src:
                        src.pop(0)()

            def ensure(*keys):
                for k in keys:
                    while k not in emitted:
                        assert qmust, f"dependency {k} not in queue"
                        qmust.pop(0)()

            def emit_S(hp, qb, kb):
                q_ch = qk_sb[hp]        # rows 0-63 head 2hp, 64-127 head 2hp+1
                k_ch = qk_sb[NPAIR + hp]
                j = kb - 4 * qb
                lo = 128 * j if j > 0 else 0
                s = ps.tile([128, 1024], dt_f32, tag="S", name="S")
                nc.tensor.matmul(
                    s[:, lo:512],
                    k_ch[0:64, kb * 128:(kb + 1) * 128],
                    q_ch[0:64, qb * 512 + lo:(qb + 1) * 512],
                    start=True, stop=True)
                nc.tensor.matmul(
                    s[:, 512 + lo:1024],
                    k_ch[64:128, kb * 128:(kb + 1) * 128],
                    q_ch[64:128, qb * 512 + lo:(qb + 1) * 512],
                    start=True, stop=True)
                pt = sb_pt.tile([128, 1024], dt_bf16, tag="PT")
                pt3 = pt[:].rearrange("p (h q) -> p h q", h=2)
                s3 = s[:].rearrange("p (h q) -> p h q", h=2)
                if lo:
                    nc.scalar.activation(
                        pt3[:, :, lo:512], s3[:, :, lo:512],
                        mybir.ActivationFunctionType.Exp, scale=INV_SQRT_D)
                else:
                    nc.scalar.activation(
                        pt[:], s[:], mybir.ActivationFunctionType.Exp,
                        scale=INV_SQRT_D)
                if j >= 0:  # triangular mask on the diagonal 128-block
                    nc.vector.tensor_mul(
                        pt3[:, :, lo:lo + 128], pt3[:, :, lo:lo + 128],
                        tri3[:])
                return pt, lo

            seg_o = {}

            def emit_O(hp, qb, kb, pt, lo):
                nkb = 4 * qb + 4
                if kb == 0:
                    seg_o[(hp, qb)] = (
                        ps.tile([D + 1, 512], dt_f32, tag="o0", name="o0",
                                bufs=1),
                        ps.tile([D + 1, 512], dt_f32, tag="o1", name="o1",
                                bufs=1))
                o0, o1 = seg_o[(hp, qb)]
                v3 = vaug_sb[kb]
                nc.tensor.matmul(
                    o0[:, lo:512],
                    v3[:, (2 * hp) * (D + 1):(2 * hp + 1) * (D + 1)],
                    pt[:, lo:512],
                    start=(kb == 0), stop=(kb == nkb - 1))
                nc.tensor.matmul(
                    o1[:, lo:512],
                    v3[:, (2 * hp + 1) * (D + 1):(2 * hp + 2) * (D + 1)],
                    pt[:, 512 + lo:1024],
                    start=(kb == 0), stop=(kb == nkb - 1))

            def emit_segend(hp, qb):
                # evict unnormalized O (one head on ScalarE, one on VectorE),
                # gather the denominator psum rows, one batched fast
                # reciprocal; broadcast+multiply deferred via nrm_q
                o0, o1 = seg_o.pop((hp, qb))
                coll = sb_nrm.tile([33, 512], dt_f32, tag="coll", name="coll")
                nc.vector.memset(coll[:], 1.0)
                stgs = []
                for h01, o in ((0, o0), (1, o1)):
                    stg = sb_stg.tile([D, 512], dt_bf16, tag="stg", name="stg")
                    nc.scalar.copy(stg[:], o[0:D, :])
                    nc.vector.tensor_copy(coll[32 * h01:32 * h01 + 1, :],
                                          o[D:D + 1, :])
                    stgs.append(stg)
                rcp = sb_nrm.tile([33, 512], dt_f32, tag="rcp", name="rcp")
                nc.vector.reciprocal_approx_fast(rcp[:], coll[:])

                def nrm_thunk(hp=hp, qb=qb, stgs=stgs, rcp=rcp):
                    for h01 in range(2):
                        rrow = sb_nrm.tile([1, 512], dt_bf16, tag="rrow",
                                           name="rrow")
                        nc.vector.tensor_copy(rrow[:],
                                              rcp[32 * h01:32 * h01 + 1, :])
                        rbc = sb_nrm.tile([64, 512], dt_bf16, tag="rbc",
                                          name="rbc")
                        nc.gpsimd.partition_broadcast(rbc[:], rrow[:])
                        nc.vector.tensor_mul(
                            ot_sb[hp][h01 * 64:(h01 + 1) * 64,
                                      qb * 512:(qb + 1) * 512],
                            stgs[h01][:], rbc[:])
                nrm_q.append(nrm_thunk)

            # ---- fill the filler queue in need-order ----
            qmust += [th for ti in range(4) for th in v_thunks(ti)]
            qmust += qk_thunks(1, 0) + qk_thunks(NPAIR + 1, 0)
            qmust += qk_thunks(2, 0) + qk_thunks(NPAIR + 2, 0)
            qmust += qk_thunks(3, 0) + qk_thunks(NPAIR + 3, 0)
            for tb in range(1, NTB):
                qmust += qk_thunks(0, tb) + qk_thunks(NPAIR, tb)
                qmust += v_thunks(4 * tb) + v_thunks(4 * tb + 1)
                qmust += v_thunks(4 * tb + 2) + v_thunks(4 * tb + 3)
                qmust += qk_thunks(1, tb) + qk_thunks(NPAIR + 1, tb)
                qmust += qk_thunks(2, tb) + qk_thunks(NPAIR + 2, tb)
                qmust += qk_thunks(3, tb) + qk_thunks(NPAIR + 3, tb)

            if lora:
                for tb in range(NTB):
                    lora_v_block(tb)
            for th in qk_thunks(0, 0) + qk_thunks(NPAIR, 0):
                th()

            steps = [(hp, qb, kb)
                     for qb in range(NTB)
                     for hp in range(NPAIR)
                     for kb in range(4 * qb + 4)]

            pend = None          # (hp, qb, kb, pt, lo) awaiting its O
            for idx, (hp, qb, kb) in enumerate(steps):
                if kb == 0:
                    ensure(("qk", hp, qb), ("qk", NPAIR + hp, qb),
                           ("v", 4 * qb + 3))
                pt, lo = emit_S(hp, qb, kb)
                if pend is not None:
                    phk, pqb, pkb, ppt, plo = pend
                    emit_O(phk, pqb, pkb, ppt, plo)
                    if pkb == 4 * pqb + 3:
                        emit_segend(phk, pqb)
                if hp == 0 and kb == 0 and qb > 0:
                    # round qb starts (previous round's last segend just
                    # emitted above): flush round qb-1's normalize, then
                    # queue its projection (reads normalized ot columns)
                    while nrm_q:
                        nrm_q.pop(0)()
                    if lora:
                        qproj.append(lambda tb=qb - 1: u_block(tb))
                    for ti in range(4 * (qb - 1), 4 * qb):
                        qproj += y_thunks(ti)
                if nrm_q and kb == 1:
                    nrm_q.pop(0)()
                if kb % 2 == 1:
                    pump(2 if 2 * (len(qmust) + len(qproj)) >
                         (len(steps) - idx) else 1)
                pend = (hp, qb, kb, pt, lo)
            phk, pqb, pkb, ppt, plo = pend
            emit_O(phk, pqb, pkb, ppt, plo)
            emit_segend(phk, pqb)

            # tail: last round's normalize + projection
            while nrm_q:
                nrm_q.pop(0)()
            if lora:
                u_block(NTB - 1)
            for i, ti in enumerate(range(4 * (NTB - 1), 4 * NTB)):
                tags = ("o0", "o1") if i >= 2 else ("pm", "pm")
                for th in y_thunks(ti, tags=tags):
                    th()
            while qmust or qproj:
                pump(1)


def build_bass(lora=True):
    nc = bacc.Bacc("TRN2", target_bir_lowering=False, debug=False, num_devices=8)
    _build_program(nc, lora=lora)
    nc.compile()
    return nc


def prepare_core_inputs(x, Wqkv, bqkv, Aqkv, Bqkv, Wproj, bproj, Aproj, Bproj):
    """Shard + lay out inputs for the 8 cores. Core c = (b = c//2, hg = c%2)."""
    def b16(a):
        return np.ascontiguousarray(a, dtype=np.float32).astype(BF16)

    # one triangular diagonal-block mask (same for every crossing offset),
    # duplicated for the two packed heads: mask[k', u] = (u >= k')
    kk = np.arange(128)[:, None]
    uu = np.arange(128)[None, :]
    m = (uu >= kk).astype(np.float32)
    masks = b16(np.concatenate([m, m], axis=1))  # [128, 256]

    aqkvT = b16(Aqkv.T)                      # [C, R]

    in_maps = []
    for c in range(8):
        bb, hg = c // 2, c % 2
        fsl = slice(hg * FV, (hg + 1) * FV)  # local head feature slice
        q_rows = slice(hg * FV, (hg + 1) * FV)
        k_rows = slice(C + hg * FV, C + (hg + 1) * FV)
        v_rows = slice(2 * C + hg * FV, 2 * C + (hg + 1) * FV)

        Wqk = np.concatenate([Wqkv[q_rows], Wqkv[k_rows]], axis=0)   # [FQK, C]
        Bqk = np.concatenate([Bqkv[q_rows], Bqkv[k_rows]], axis=0)   # [FQK, R]
        bqk = np.concatenate([bqkv[q_rows], bqkv[k_rows]], axis=0)   # [FQK]

        in_maps.append({
            "xT": b16(np.asarray(x)[bb].T),
            "wqkT": b16(Wqk.T),
            "auga_qk": b16(np.concatenate(
                [SCALE * Bqk.T, bqk[None, :]], axis=0)),
            "wvT": b16(Wqkv[v_rows].T),
            "augb_v": b16(np.concatenate(
                [SCALE * Bqkv[v_rows].T, bqkv[v_rows][None, :]], axis=0)),
            "aqkvT": aqkvT,
            "wpT": b16(Wproj[:, fsl].T),
            "apT": b16(Aproj[:, fsl].T),
            "augb_p": b16(np.concatenate(
                [SCALE * Bproj.T, 0.5 * bproj[None, :]], axis=0)),
            "masks": masks,
        })
    return in_maps


_CACHED_NC = None
TRACE = False  # set True (e.g. from test.py) to request an NTFF-profiled run


def _install_axon_ntff_hook():
    """Provide antenv.axon_hooks (NTFF profiling hook) if the image lacks it.

    Mirrors trn_agent_boot.trn_boot._ntff_profile_via_ctypes: drives NRT
    profiling on the axon terminal via the libaxon_pjrt.so C ABI.
    """
    try:
        from antenv.axon_hooks import get_axon_ntff_profile_hook  # noqa: F401
        return
    except ImportError:
        pass
    import contextlib
    import ctypes
    import types

    import antenv

    so_path = "/opt/axon/libaxon_pjrt.so"
    hook = None
    if os.path.exists(so_path):
        lib = ctypes.CDLL(so_path)
        if hasattr(lib, "axon_start_nrt_profile"):
            lib.axon_start_nrt_profile.argtypes = [
                ctypes.POINTER(ctypes.c_int64), ctypes.c_size_t]
            lib.axon_start_nrt_profile.restype = ctypes.c_int64
            lib.axon_stop_nrt_profile.argtypes = [ctypes.c_char_p]
            lib.axon_stop_nrt_profile.restype = ctypes.c_int64

            @contextlib.contextmanager
            def _hook(output_dir, device_ids):
                import jax
                jax.devices()
                if device_ids:
                    ids = (ctypes.c_int64 * len(device_ids))(*device_ids)
                    rc = lib.axon_start_nrt_profile(ids, len(device_ids))
                else:
                    rc = lib.axon_start_nrt_profile(None, 0)
                if rc != 0:
                    raise RuntimeError(f"axon_start_nrt_profile rc={rc}")
                try:
                    yield
                finally:
                    n = lib.axon_stop_nrt_profile(str(output_dir).encode())
                    print(f"ntff profile: {n} file(s) -> {output_dir}",
                          file=sys.stderr)

            hook = _hook

    mod = types.ModuleType("antenv.axon_hooks")
    state = {"h": hook}
    mod.get_axon_ntff_profile_hook = lambda: state["h"]
    mod.set_axon_ntff_profile_hook = lambda h: state.update(h=h)
    sys.modules["antenv.axon_hooks"] = mod
    antenv.axon_hooks = mod


def kernel(**inputs):
    global _CACHED_NC, LAST_RESULTS
    in_maps = prepare_core_inputs(**inputs)
    # loralib initializes B to zero, and the biases here are zero: when every
    # adapter/bias contribution is exactly zero, the extra contraction tiles
    # are mathematically a no-op — use the leaner program variant.
    lora = any(
        np.any(np.asarray(inputs[k]) != 0)
        for k in ("Bqkv", "Bproj", "bqkv", "bproj"))
    if _CACHED_NC is None:
        _CACHED_NC = build_bass(lora=lora)
    if TRACE:
        _install_axon_ntff_hook()
    res = run_bass_kernel_spmd(
        _CACHED_NC, in_maps, core_ids=list(range(8)), trace=TRACE,
    )
    LAST_RESULTS = res
    y = np.zeros((B, T, C), dtype=np.float32)
    for c in range(8):
        y[c // 2] += np.asarray(res.results[c]["out"]).astype(np.float32)
    return y
